# revision 1
# baseline (speedup 1.0000x reference)
"""Sliding-window GQA attention (Gemma-style) on 8 TRN2 NeuronCores.

Sharding: tensor-parallel over heads. Core c owns q-heads {2c, 2c+1} and
kv-head c. Each core computes Q/K/V projections (+RoPE) for its heads over
the full sequence, banded sliding-window attention, then an AllToAll
(split into 2 token-chunks, overlapped with attention/output compute)
reshards the attention output by token so every core computes the full
output projection for its 512-token slice. Host concatenates slices.

All matmuls run in bf16 (f32 PSUM accumulation); softmax runs in f32.
"""

import os
import sys

for _p in ("/opt/trn_rl_repo",):
    if _p not in sys.path:
        sys.path.insert(0, _p)

import numpy as np
import ml_dtypes

import concourse.bass as bass
import concourse.mybir as mybir
import concourse.tile as tile
from concourse import bacc
from concourse.bass_utils import run_bass_kernel_spmd
from concourse.masks import make_identity

F32 = mybir.dt.float32
BF16 = mybir.dt.bfloat16
AF = mybir.ActivationFunctionType
ALU = mybir.AluOpType

B, T, D = 2, 2048, 3584
NQ, NKV, H = 16, 8, 256
SCALAR = 0.0625
SOFT_CAP = 50.0
WINDOW = 1024
ROPE_BASE = 10000.0

NCORES = 8
P = 128
DC = D // P              # 28 contraction chunks
TQ = T // P              # 16 query tiles per batch
TPIECE = 256             # projection output tile width
NPIECE = T // TPIECE
HLOC = 2 * H             # 512 local q-head columns per core
TOK = B * T              # 4096
TPC = TOK // NCORES      # 512 tokens per core after AllToAll
CHTOK = TPC // 2         # 256 tokens per A2A chunk block
WTILES = WINDOW // P     # 8
MASKVAL = -1.0e30        # added to tanh output; exp(50*(t+MASKVAL)) == 0
SEGMAX = 8               # max QK tiles per PSUM strip segment (2 banks)
NHC = NQ * H // P        # 32 global h chunks
DP = 512                 # output projection d piece

last_result = None       # BassKernelResults of the most recent device run


def _band(i, mode):
    lo = max(0, i - WTILES)
    hi = i if mode == "tril" else min(TQ - 1, i + WTILES)
    return lo, hi


def _segments(lo, hi):
    segs = []
    j = lo
    while j <= hi:
        j1 = min(j + SEGMAX - 1, hi)
        segs.append((j, j1))
        j = j1 + 1
    return segs


def build(mode):
    assert mode in ("tril", "ones")
    nseg_max = 3 if mode == "ones" else 2
    nc = bacc.Bacc("TRN2", target_bir_lowering=False, debug=False,
                   num_devices=NCORES)

    xT = nc.dram_tensor("xT", [D, TOK], BF16, kind="ExternalInput")
    wq = nc.dram_tensor("wq", [D, HLOC], BF16, kind="ExternalInput")
    wk = nc.dram_tensor("wk", [D, H], BF16, kind="ExternalInput")
    wv = nc.dram_tensor("wv", [D, H], BF16, kind="ExternalInput")
    wo = nc.dram_tensor("wo", [NHC, P, D], BF16, kind="ExternalInput")
    ropeq = nc.dram_tensor("ropeq", [2, P, T], BF16, kind="ExternalInput")
    ropek = nc.dram_tensor("ropek", [2, P, T], BF16, kind="ExternalInput")
    out = nc.dram_tensor("out", [TPC, D], F32, kind="ExternalOutput")

    with tile.TileContext(nc) as tc:
        with (
            tc.tile_pool(name="dram", bufs=1, space="DRAM") as dram,
            tc.tile_pool(name="consts", bufs=1) as consts,
            tc.tile_pool(name="qkv", bufs=1) as qkvpool,
        ):
            # A2A bounce buffers, token-major: [src_rank_block][tok][local h]
            a2a_in = [dram.tile([NCORES, HLOC, CHTOK], BF16,
                                name=f"a2a_in{m}") for m in range(2)]
            a2a_out = [dram.tile([NCORES, HLOC, CHTOK], BF16,
                                 name=f"a2a_out{m}") for m in range(2)]

            # ---- constants ----
            ident = consts.tile([P, P], F32)
            make_identity(nc, ident)
            # causal: valid (0) where k <= q, MASKVAL above diag
            causal = consts.tile([P, P], F32)
            nc.gpsimd.memset(causal, 0.0)
            nc.gpsimd.affine_select(
                out=causal, in_=causal, compare_op=ALU.is_ge, fill=MASKVAL,
                base=0, pattern=[[-1, P]], channel_multiplier=1)
            # upperstrict: valid (0) where q < k (window lower edge, j=i-8)
            upperstrict = consts.tile([P, P], F32)
            nc.gpsimd.memset(upperstrict, MASKVAL)
            nc.gpsimd.affine_select(
                out=upperstrict, in_=upperstrict, compare_op=ALU.is_ge,
                fill=0.0, base=0, pattern=[[-1, P]], channel_multiplier=1)
            # lowerstrict: valid (0) where k < q (window upper edge, j=i+8)
            lowerstrict = None
            if mode == "ones":
                lowerstrict = consts.tile([P, P], F32)
                nc.gpsimd.memset(lowerstrict, 0.0)
                nc.gpsimd.affine_select(
                    out=lowerstrict, in_=lowerstrict, compare_op=ALU.is_gt,
                    fill=MASKVAL, base=0, pattern=[[-1, P]],
                    channel_multiplier=1)

            qts, kts, vsbs = [], [], []
            xT_v = xT[:].rearrange("(c p) t -> p c t", p=P)

            # =================== projections + rope =======================
            with (
                tc.tile_pool(name="weights", bufs=1) as wpool,
                tc.tile_pool(name="xt", bufs=2) as xpool,
                tc.tile_pool(name="rtmp", bufs=2) as rpool,
                tc.tile_pool(name="proj_ps", bufs=4, space="PSUM") as ppsum,
                tc.tile_pool(name="projv_ps", bufs=2, space="PSUM") as vpsum,
            ):
                wq_sb = wpool.tile([P, DC, HLOC], BF16)
                wq_v = wq[:].rearrange("(c p) h -> p c h", p=P)
                nc.sync.dma_start(wq_sb[:, :DC // 2, :], wq_v[:, :DC // 2, :])
                nc.sync.dma_start(wq_sb[:, DC // 2:, :], wq_v[:, DC // 2:, :])
                wk_sb = wpool.tile([P, DC, H], BF16)
                nc.sync.dma_start(wk_sb[:],
                                  wk[:].rearrange("(c p) h -> p c h", p=P))
                wv_sb = wpool.tile([P, DC, H], BF16)
                nc.sync.dma_start(wv_sb[:],
                                  wv[:].rearrange("(c p) h -> p c h", p=P))
                rq_sb = wpool.tile([P, 2, T], BF16)
                nc.sync.dma_start(rq_sb[:], ropeq[:].rearrange("s p t -> p s t"))
                rk_sb = wpool.tile([P, 2, T], BF16)
                nc.sync.dma_start(rk_sb[:], ropek[:].rearrange("s p t -> p s t"))

                for b in range(B):
                    qt = qkvpool.tile([P, 4, T], BF16, tag=f"qt{b}")
                    kt = qkvpool.tile([P, 2, T], BF16, tag=f"kt{b}")
                    vsb = qkvpool.tile([P, TQ, H], BF16, tag=f"v{b}")
                    qts.append(qt)
                    kts.append(kt)
                    vsbs.append(vsb)

                    for pi in range(NPIECE):
                        t0 = pi * TPIECE
                        xt = xpool.tile([P, DC, TPIECE], BF16, tag="xt")
                        nc.sync.dma_start(
                            xt[:], xT_v[:, :, b * T + t0:b * T + t0 + TPIECE])

                        def rope(dst, hc0, psA, psB, tab, t0=t0):
                            cos = tab[:, 0, t0:t0 + TPIECE]
                            sin = tab[:, 1, t0:t0 + TPIECE]
                            t1 = rpool.tile([P, TPIECE], F32, tag="r1")
                            t2 = rpool.tile([P, TPIECE], F32, tag="r2")
                            nc.vector.tensor_tensor(t1[:], psA[:], cos, ALU.mult)
                            nc.vector.tensor_tensor(t2[:], psB[:], sin, ALU.mult)
                            nc.vector.tensor_tensor(
                                dst[:, hc0, t0:t0 + TPIECE], t1[:], t2[:],
                                ALU.subtract)
                            t3 = rpool.tile([P, TPIECE], F32, tag="r3")
                            t4 = rpool.tile([P, TPIECE], F32, tag="r4")
                            nc.vector.tensor_tensor(t3[:], psB[:], cos, ALU.mult)
                            nc.vector.tensor_tensor(t4[:], psA[:], sin, ALU.mult)
                            nc.vector.tensor_tensor(
                                dst[:, hc0 + 1, t0:t0 + TPIECE], t3[:], t4[:],
                                ALU.add)

                        # Q projections: 4 h-chunks (2 heads x 2 halves)
                        for hh in range(2):
                            ps = []
                            for half in range(2):
                                hc = hh * 2 + half
                                pq = ppsum.tile([P, TPIECE], F32, tag="pq")
                                for dc in range(DC):
                                    nc.tensor.matmul(
                                        pq[:],
                                        wq_sb[:, dc, hc * P:(hc + 1) * P],
                                        xt[:, dc, :],
                                        start=(dc == 0), stop=(dc == DC - 1))
                                ps.append(pq)
                            rope(qt, hh * 2, ps[0], ps[1], rq_sb)
                        # K projection: 2 h-chunks
                        ps = []
                        for half in range(2):
                            pk = ppsum.tile([P, TPIECE], F32, tag="pq")
                            for dc in range(DC):
                                nc.tensor.matmul(
                                    pk[:],
                                    wk_sb[:, dc, half * P:(half + 1) * P],
                                    xt[:, dc, :],
                                    start=(dc == 0), stop=(dc == DC - 1))
                            ps.append(pk)
                        rope(kt, 0, ps[0], ps[1], rk_sb)
                        # V projection: natural layout [t, h]
                        for tc4 in range(TPIECE // P):
                            pv = vpsum.tile([P, H], F32, tag="pv")
                            for dc in range(DC):
                                nc.tensor.matmul(
                                    pv[:],
                                    xt[:, dc, tc4 * P:(tc4 + 1) * P],
                                    wv_sb[:, dc, :],
                                    start=(dc == 0), stop=(dc == DC - 1))
                            nc.vector.tensor_copy(
                                out=vsb[:, pi * (TPIECE // P) + tc4, :],
                                in_=pv[:])

            # ============ banded attention in 2 rounds + A2A ==============
            with (
                tc.tile_pool(name="attn_sb", bufs=2) as apool,
                tc.tile_pool(name="stats", bufs=3) as spool,
                tc.tile_pool(name="attn_ps", bufs=2, space="PSUM") as apsum,
                tc.tile_pool(name="tr_ps", bufs=1, space="PSUM") as trpsum,
                tc.tile_pool(name="enc_ps", bufs=1, space="PSUM") as encpsum,
            ):
                def attend(b, hh, i):
                    qt, kt, vsb = qts[b], kts[b], vsbs[b]
                    lo, hi = _band(i, mode)
                    segs = _segments(lo, hi)
                    tstrips, rsums = [], []
                    for si, (j0, j1) in enumerate(segs):
                        nj = j1 - j0 + 1
                        w = nj * P
                        qk = apsum.tile([P, SEGMAX * P], F32, tag="qk")
                        # QK^T in 512-wide blocks, contiguous group per block
                        for blk0 in range(0, w, 512):
                            bw = min(512, w - blk0)
                            for c in range(2):
                                nc.tensor.matmul(
                                    qk[:, blk0:blk0 + bw],
                                    qt[:, hh * 2 + c, i * P:(i + 1) * P],
                                    kt[:, c,
                                       j0 * P + blk0:j0 * P + blk0 + bw],
                                    start=(c == 0), stop=(c == 1))
                        ts_ = apool.tile([P, w], F32, tag=f"tstrip{si}")
                        nc.scalar.activation(ts_[:, :w], qk[:, :w], AF.Tanh,
                                             scale=1.0 / SOFT_CAP)
                        for jj in range(nj):
                            j = j0 + jj
                            m = None
                            if j == i - WTILES:
                                m = upperstrict
                            elif j == i and mode == "tril":
                                m = causal
                            elif j == i + WTILES and mode == "ones":
                                m = lowerstrict
                            if m is not None:
                                sl = slice(jj * P, (jj + 1) * P)
                                nc.vector.tensor_tensor(
                                    ts_[:, sl], ts_[:, sl], m[:], ALU.add)
                        rs = spool.tile([P, 1], F32, tag=f"rs{si}")
                        nc.scalar.activation(ts_[:, :w], ts_[:, :w], AF.Exp,
                                             scale=SOFT_CAP, accum_out=rs[:])
                        tstrips.append(ts_)
                        rsums.append(rs)
                    rtot = rsums[0]
                    for si in range(1, len(rsums)):
                        nrt = spool.tile([P, 1], F32, tag=f"rtot{si}")
                        nc.vector.tensor_tensor(nrt[:], rtot[:],
                                                rsums[si][:], ALU.add)
                        rtot = nrt
                    rinv = spool.tile([P, 1], F32, tag="rinv")
                    nc.vector.reciprocal(rinv[:], rtot[:])
                    # normalize -> transpose (f32 PE path)
                    pts = []
                    for si, (j0, j1) in enumerate(segs):
                        nj = j1 - j0 + 1
                        w = nj * P
                        pn = apool.tile([P, w], F32, tag=f"pn{si}")
                        nc.vector.tensor_scalar_mul(
                            pn[:, :w], tstrips[si][:, :w], rinv[:])
                        tr = trpsum.tile([P, SEGMAX * P], F32, tag="tr")
                        for jj in range(nj):
                            nc.tensor.transpose(tr[:, jj * P:(jj + 1) * P],
                                                pn[:, jj * P:(jj + 1) * P],
                                                ident[:])
                        pt = apool.tile([P, w], BF16, tag=f"pt{si}")
                        nc.vector.tensor_copy(out=pt[:, :w], in_=tr[:, :w])
                        pts.append(pt)
                    # PV: enc^T [h, q]; contiguous accumulation group per half
                    njs = [j1 - j0 + 1 for j0, j1 in segs]
                    ntot = sum(njs)
                    encsb = apool.tile([P, 2, P], BF16, tag="encsb")
                    for c in range(2):
                        encc = encpsum.tile([P, P], F32, tag=f"enc{c}")
                        n = 0
                        for si, (j0, j1) in enumerate(segs):
                            for jj in range(njs[si]):
                                j = j0 + jj
                                nc.tensor.matmul(
                                    encc[:],
                                    vsb[:, j, c * P:(c + 1) * P],
                                    pts[si][:, jj * P:(jj + 1) * P],
                                    start=(n == 0), stop=(n == ntot - 1))
                                n += 1
                        nc.vector.tensor_copy(out=encsb[:, c, :],
                                              in_=encc[:])
                    gtok = b * T + i * P
                    jb = gtok // TPC
                    ch = (gtok % TPC) // CHTOK
                    toff = gtok % CHTOK
                    dst = a2a_in[ch][jb].rearrange(
                        "(c p) t -> p c t",
                        p=P)[:, 2 * hh:2 * hh + 2, toff:toff + P]
                    nc.sync.dma_start(dst, encsb[:])

                for ch in range(2):
                    for b in range(B):
                        for hh in range(2):
                            for i in range(TQ):
                                if (i % 4) // 2 == ch:
                                    attend(b, hh, i)
                    nc.gpsimd.collective_compute(
                        "AllToAll", ALU.bypass,
                        replica_groups=[list(range(NCORES))],
                        ins=[a2a_in[ch][:].opt()],
                        outs=[a2a_out[ch][:].opt()])

            with (
                tc.tile_pool(name="oproj", bufs=2) as opool,
                tc.tile_pool(name="enc_full", bufs=1) as efpool,
                tc.tile_pool(name="oproj_ps", bufs=2, space="PSUM") as opsum,
            ):
                efs = []
                for ch in range(2):
                    efc = efpool.tile([P, NHC, CHTOK], BF16, tag=f"ef{ch}")
                    nc.sync.dma_start(
                        efc[:],
                        a2a_out[ch][:].rearrange("b (c p) t -> p (b c) t",
                                                 p=P))
                    efs.append(efc)
                for dp in range(D // DP):
                    wo_sb = opool.tile([P, NHC, DP], BF16, tag="wo")
                    nc.sync.dma_start(
                        wo_sb[:],
                        wo[:, :, dp * DP:(dp + 1) * DP].rearrange(
                            "c p d -> p c d"))
                    for tc4 in range(TPC // P):
                        ef = efs[tc4 // 2]
                        tc2 = tc4 % 2
                        po = opsum.tile([P, DP], F32, tag="po")
                        for hc in range(NHC):
                            nc.tensor.matmul(
                                po[:], ef[:, hc, tc2 * P:(tc2 + 1) * P],
                                wo_sb[:, hc, :],
                                start=(hc == 0), stop=(hc == NHC - 1))
                        osb = opool.tile([P, DP], F32, tag="osb")
                        nc.vector.tensor_copy(out=osb[:], in_=po[:])
                        nc.sync.dma_start(
                            out[tc4 * P:(tc4 + 1) * P, dp * DP:(dp + 1) * DP],
                            osb[:])

    nc.compile()
    return nc


def _rope_tables(pos, scale):
    """pos: [T] int array -> [2, 128, T] bf16 (cos;sin), scaled."""
    frac = 2.0 * np.arange(H // 2, dtype=np.float64) / H
    timescale = ROPE_BASE ** frac                      # [128]
    sinusoid = pos.astype(np.float64)[None, :] / timescale[:, None]  # [128,T]
    tabs = np.stack([np.cos(sinusoid), np.sin(sinusoid)]) * scale
    return tabs.astype(ml_dtypes.bfloat16)


def _reference_host(x, segment_pos, attn_mask, w_q, w_kv, w_o):
    """Slow but fully general fallback (numpy)."""
    xb = x.astype(np.float32)
    q = np.einsum('btd,ndh->btnh', xb, w_q)
    k = np.einsum('bsd,kdh->bskh', xb, w_kv[0])
    v = np.einsum('bsd,kdh->bskh', xb, w_kv[1])

    def rope(t, positions):
        hd = t.shape[-1]
        frac = 2.0 * np.arange(hd // 2, dtype=np.float32) / hd
        ts_ = ROPE_BASE ** frac
        sinusoid = positions.astype(np.float32)[..., None] / ts_
        sinusoid = sinusoid[..., None, :]
        s, c = np.sin(sinusoid), np.cos(sinusoid)
        first, second = np.split(t, 2, axis=-1)
        return np.concatenate([first * c - second * s,
                               second * c + first * s], axis=-1)

    q = rope(q, segment_pos) * SCALAR
    k = rope(k, segment_pos)
    qg = q.reshape(B, T, NKV, 2, H)
    logits = np.einsum('btkgh,bskh->btkgs', qg, k).reshape(B, T, NQ, T)
    logits = np.tanh(logits / SOFT_CAP) * SOFT_CAP
    pos_s = np.arange(T)[None, None, :]
    pos_t = segment_pos[:, :, None]
    sliding = (pos_s > pos_t - WINDOW) & (pos_s < pos_t + WINDOW)
    mask = np.logical_and(attn_mask, sliding)
    padded = np.where(mask[:, :, None, :], logits, -np.inf)
    padded -= padded.max(axis=-1, keepdims=True)
    e = np.exp(padded)
    probs = (e / e.sum(axis=-1, keepdims=True)).astype(np.float32)
    v_exp = np.repeat(v, NQ // NKV, axis=2)            # [B,T,NQ,H]
    enc = np.einsum('btns,bsnh->btnh', probs, v_exp)
    return np.einsum('btnh,nhd->btd', enc, w_o).astype(np.float32)


_GRAPH_CACHE = {}


def kernel(x, segment_pos, attn_mask, w_q, w_kv, w_o):
    global last_result
    x = np.asarray(x)
    segment_pos = np.asarray(segment_pos)
    attn_mask = np.asarray(attn_mask)
    w_q = np.asarray(w_q, dtype=np.float32)
    w_kv = np.asarray(w_kv, dtype=np.float32)
    w_o = np.asarray(w_o, dtype=np.float32)

    arange = np.broadcast_to(np.arange(T, dtype=segment_pos.dtype), (B, T))
    std_pos = np.array_equal(segment_pos, arange)
    tril = np.broadcast_to(np.tril(np.ones((T, T), dtype=bool)), (B, T, T))
    if attn_mask.all():
        mode = "ones"
    elif np.array_equal(attn_mask, tril):
        mode = "tril"
    else:
        mode = None
    if not std_pos or mode is None:
        return _reference_host(x, segment_pos, attn_mask, w_q, w_kv, w_o)

    if mode not in _GRAPH_CACHE:
        _GRAPH_CACHE[mode] = build(mode)
    nc = _GRAPH_CACHE[mode]

    bf = ml_dtypes.bfloat16
    xT = np.ascontiguousarray(x.reshape(TOK, D).T).astype(bf)    # [D, TOK]
    pos = segment_pos[0]
    ropeq = np.ascontiguousarray(_rope_tables(pos, SCALAR))
    ropek = np.ascontiguousarray(_rope_tables(pos, 1.0))
    wo_all = np.ascontiguousarray(
        w_o.reshape(NHC, P, D)).astype(bf)

    in_maps = []
    for c in range(NCORES):
        wq_c = np.ascontiguousarray(
            np.concatenate([w_q[2 * c], w_q[2 * c + 1]], axis=1)).astype(bf)
        wk_c = np.ascontiguousarray(w_kv[0, c]).astype(bf)
        wv_c = np.ascontiguousarray(w_kv[1, c]).astype(bf)
        in_maps.append({
            "xT": xT, "wq": wq_c, "wk": wk_c, "wv": wv_c, "wo": wo_all,
            "ropeq": ropeq, "ropek": ropek,
        })

    trace = os.environ.get("KTRACE", "0") == "1"
    res = run_bass_kernel_spmd(nc, in_maps, core_ids=list(range(NCORES)),
                               trace=trace)
    last_result = res
    outs = [res.results[c]["out"] for c in range(NCORES)]
    return np.concatenate(outs, axis=0).reshape(B, T, D).astype(np.float32)



# revision 11
# speedup vs baseline: 1.0673x; 1.0673x over previous
"""Sliding-window GQA attention (Gemma-style) on 8 TRN2 NeuronCores.

Sharding: tensor-parallel over heads. Core c owns q-heads {2c, 2c+1} and
kv-head c. Each core computes Q/K/V projections (+RoPE) for its heads over
the full sequence, banded sliding-window attention, then an AllToAll
(split into 2 token-chunks) reshards the attention output by token so
every core computes the full output projection for its 512-token slice.
Host concatenates slices.

Attention uses a [k, q] logits layout (K stationary, Q moving) so the
softmax probabilities come out of the Scalar engine directly in the
[k, q] orientation PV needs -- no PE transposes and no per-strip
normalization passes on the Vector engine. Row sums (softmax
denominators) are computed with ones-vector matmuls accumulated in PSUM;
the normalization is applied once on the (much smaller) attention output
via a broadcast matmul + Reciprocal activation.

All matmuls run in bf16 (f32 PSUM accumulation); softmax runs in f32.
"""

import os
import sys

for _p in ("/opt/trn_rl_repo",):
    if _p not in sys.path:
        sys.path.insert(0, _p)

import numpy as np
import ml_dtypes

import concourse.bass as bass
import concourse.mybir as mybir
import concourse.tile as tile
from concourse import bacc
from concourse.bass_utils import run_bass_kernel_spmd

F32 = mybir.dt.float32
BF16 = mybir.dt.bfloat16
AF = mybir.ActivationFunctionType
ALU = mybir.AluOpType

B, T, D = 2, 2048, 3584
NQ, NKV, H = 16, 8, 256
SCALAR = 0.0625
SOFT_CAP = 50.0
WINDOW = 1024
ROPE_BASE = 10000.0

NCORES = 8
P = 128
DC = D // P              # 28 contraction chunks
TQ = T // P              # 16 query tiles per batch
TPIECE = 256             # projection output tile width
NPIECE = T // TPIECE
HLOC = 2 * H             # 512 local q-head columns per core
TOK = B * T              # 4096
TPC = TOK // NCORES      # 512 tokens per core after AllToAll
CHTOK = TPC // 2         # 256 tokens per A2A chunk block
WTILES = WINDOW // P     # 8
MASKVAL = -1.0e30        # added to tanh output; exp(50*(t+MASKVAL)) == 0
NHC = NQ * H // P        # 32 global h chunks
DP = 512                 # output projection d piece

last_result = None       # BassKernelResults of the most recent device run


def _band(i, mode):
    lo = max(0, i - WTILES)
    hi = i if mode == "tril" else min(TQ - 1, i + WTILES)
    return lo, hi


def build(mode):
    assert mode in ("tril", "ones")
    nc = bacc.Bacc("TRN2", target_bir_lowering=False, debug=False,
                   num_devices=NCORES)

    xT = nc.dram_tensor("xT", [D, TOK], BF16, kind="ExternalInput")
    wq = nc.dram_tensor("wq", [D, HLOC], BF16, kind="ExternalInput")
    wk = nc.dram_tensor("wk", [D, H], BF16, kind="ExternalInput")
    wv = nc.dram_tensor("wv", [D, H], BF16, kind="ExternalInput")
    wo = nc.dram_tensor("wo", [NHC, P, D], BF16, kind="ExternalInput")
    ropeq = nc.dram_tensor("ropeq", [2, P, T], BF16, kind="ExternalInput")
    ropek = nc.dram_tensor("ropek", [2, P, T], BF16, kind="ExternalInput")
    msk = nc.dram_tensor("msk", [3, P, 2 * P], F32, kind="ExternalInput")
    out = nc.dram_tensor("out", [TPC, D], F32, kind="ExternalOutput")

    with tile.TileContext(nc) as tc:
        with (
            tc.tile_pool(name="dram", bufs=1, space="DRAM") as dram,
            tc.tile_pool(name="consts", bufs=1) as consts,
            tc.tile_pool(name="qkv", bufs=1) as qkvpool,
        ):
            # A2A bounce buffers: [src_rank_block][local h chunk-major][tok]
            a2a_in = [dram.tile([NCORES, HLOC, CHTOK], BF16,
                                name=f"a2a_in{m}") for m in range(2)]
            a2a_out = [dram.tile([NCORES, HLOC, CHTOK], BF16,
                                 name=f"a2a_out{m}") for m in range(2)]

            # ---- constants ----
            ones_col = consts.tile([P, 1], BF16)
            nc.gpsimd.memset(ones_col, 1.0)
            msk_sb = consts.tile([P, 3, 2 * P], F32)

            qts, kts, vsbs = [], [], []
            xT_v = xT[:].rearrange("(c p) t -> p c t", p=P)

            # =================== projections + rope =======================
            # qt chunk order is (c*2 + h): [c0h0, c0h1, c1h0, c1h1] so the
            # QK matmul rhs for contraction chunk c is a contiguous slice.
            with (
                tc.tile_pool(name="weights", bufs=1) as wpool,
                tc.tile_pool(name="xt", bufs=2) as xpool,
                tc.tile_pool(name="rtmp", bufs=2) as rpool,
                tc.tile_pool(name="proj_ps", bufs=4, space="PSUM") as ppsum,
                tc.tile_pool(name="projv_ps", bufs=2, space="PSUM") as vpsum,
            ):
                # Staged weight loads: first slices small so the first
                # matmuls can start within a few us of kernel start.
                wq_sb = wpool.tile([P, DC, HLOC], BF16)
                wq_v = wq[:].rearrange("(c p) h -> p c h", p=P)
                for d0, d1 in ((0, 4), (4, 12), (12, 20), (20, DC)):
                    nc.sync.dma_start(wq_sb[:, d0:d1, :], wq_v[:, d0:d1, :])
                rq_sb = wpool.tile([P, 2, T], BF16)
                nc.sync.dma_start(rq_sb[:], ropeq[:].rearrange("s p t -> p s t"))
                wk_sb = wpool.tile([P, DC, H], BF16)
                nc.sync.dma_start(wk_sb[:],
                                  wk[:].rearrange("(c p) h -> p c h", p=P))
                wv_sb = wpool.tile([P, DC, H], BF16)
                nc.sync.dma_start(wv_sb[:],
                                  wv[:].rearrange("(c p) h -> p c h", p=P))
                rk_sb = wpool.tile([P, 2, T], BF16)
                nc.sync.dma_start(rk_sb[:], ropek[:].rearrange("s p t -> p s t"))
                nc.sync.dma_start(msk_sb[:], msk[:].rearrange("m p q -> p m q"))

                for b in range(B):
                    qt = qkvpool.tile([P, 4, T], BF16, tag=f"qt{b}")
                    kt = qkvpool.tile([P, 2, T], BF16, tag=f"kt{b}")
                    vsb = qkvpool.tile([P, TQ, H], BF16, tag=f"v{b}")
                    qts.append(qt)
                    kts.append(kt)
                    vsbs.append(vsb)

                    for pi in range(NPIECE):
                        t0 = pi * TPIECE
                        xt = xpool.tile([P, DC, TPIECE], BF16, tag="xt")
                        src = xT_v[:, :, b * T + t0:b * T + t0 + TPIECE]
                        if b == 0 and pi == 0:
                            nc.sync.dma_start(xt[:, :4, :], src[:, :4, :])
                            nc.sync.dma_start(xt[:, 4:, :], src[:, 4:, :])
                        else:
                            nc.sync.dma_start(xt[:], src)

                        def rope(dst, i0, i1, psA, psB, tab, t0=t0):
                            cos = tab[:, 0, t0:t0 + TPIECE]
                            sin = tab[:, 1, t0:t0 + TPIECE]
                            t1 = rpool.tile([P, TPIECE], F32, tag="r1")
                            t2 = rpool.tile([P, TPIECE], F32, tag="r2")
                            nc.vector.tensor_tensor(t1[:], psA[:], cos, ALU.mult)
                            nc.vector.tensor_tensor(t2[:], psB[:], sin, ALU.mult)
                            nc.vector.tensor_tensor(
                                dst[:, i0, t0:t0 + TPIECE], t1[:], t2[:],
                                ALU.subtract)
                            t3 = rpool.tile([P, TPIECE], F32, tag="r3")
                            t4 = rpool.tile([P, TPIECE], F32, tag="r4")
                            nc.vector.tensor_tensor(t3[:], psB[:], cos, ALU.mult)
                            nc.vector.tensor_tensor(t4[:], psA[:], sin, ALU.mult)
                            nc.vector.tensor_tensor(
                                dst[:, i1, t0:t0 + TPIECE], t3[:], t4[:],
                                ALU.add)

                        # Q projections: head h -> chunks h (c0) and 2+h (c1)
                        for hh in range(2):
                            ps = []
                            for cc in range(2):
                                hc = cc * 2 + hh
                                pq = ppsum.tile([P, TPIECE], F32, tag="pq")
                                for dc in range(DC):
                                    nc.tensor.matmul(
                                        pq[:],
                                        wq_sb[:, dc, hc * P:(hc + 1) * P],
                                        xt[:, dc, :],
                                        start=(dc == 0), stop=(dc == DC - 1))
                                ps.append(pq)
                            rope(qt, hh, 2 + hh, ps[0], ps[1], rq_sb)
                        # K projection: 2 h-chunks
                        ps = []
                        for half in range(2):
                            pk = ppsum.tile([P, TPIECE], F32, tag="pq")
                            for dc in range(DC):
                                nc.tensor.matmul(
                                    pk[:],
                                    wk_sb[:, dc, half * P:(half + 1) * P],
                                    xt[:, dc, :],
                                    start=(dc == 0), stop=(dc == DC - 1))
                            ps.append(pk)
                        rope(kt, 0, 1, ps[0], ps[1], rk_sb)
                        # V projection: natural layout [t, h]
                        for tc4 in range(TPIECE // P):
                            pv = vpsum.tile([P, H], F32, tag="pv")
                            for dc in range(DC):
                                nc.tensor.matmul(
                                    pv[:],
                                    xt[:, dc, tc4 * P:(tc4 + 1) * P],
                                    wv_sb[:, dc, :],
                                    start=(dc == 0), stop=(dc == DC - 1))
                            nc.vector.tensor_copy(
                                out=vsb[:, pi * (TPIECE // P) + tc4, :],
                                in_=pv[:])

            # ============ banded attention in 2 rounds + A2A ==============
            # Output projection SBUF pools open early so wo prefetch can
            # start during the attention phase.
            with (
                tc.tile_pool(name="wo_sb", bufs=2) as wopool,
                tc.tile_pool(name="ef", bufs=1) as efpool,
                tc.tile_pool(name="osb", bufs=3) as ospool,
            ):
                wo_tiles = {}

                def load_wo(half, dp):
                    t = wopool.tile([P, NHC, DP], BF16, tag="wo")
                    nc.sync.dma_start(
                        t[:],
                        wo[:, :, dp * DP:(dp + 1) * DP].rearrange(
                            "c p d -> p c d"))
                    wo_tiles[(half, dp)] = t

                efs = {}

                def load_ef(half):
                    ef = efpool.tile([P, NHC, CHTOK], BF16, tag=f"ef{half}")
                    nc.gpsimd.dma_start(
                        ef[:],
                        a2a_out[half][:].rearrange("b (c p) t -> p (b c) t",
                                                   p=P))
                    efs[half] = ef

                with (
                    tc.tile_pool(name="attn_sb", bufs=3) as apool,
                    tc.tile_pool(name="es_sb", bufs=2) as espool,
                    tc.tile_pool(name="rb_sb", bufs=2) as rbpool,
                    tc.tile_pool(name="qk_ps", bufs=3, space="PSUM") as qkps,
                    tc.tile_pool(name="dn_ps", bufs=2, space="PSUM") as dnps,
                    tc.tile_pool(name="enc_ps", bufs=1, space="PSUM") as encps,
                ):
                    def attend(b, i):
                        qt, kt, vsb = qts[b], kts[b], vsbs[b]
                        lo, hi = _band(i, mode)
                        nj = hi - lo + 1
                        es_list = []
                        for j in range(lo, hi + 1):
                            qk = qkps.tile([P, 2 * P], F32, tag="qk")
                            for c in range(2):
                                nc.tensor.matmul(
                                    qk[:],
                                    kt[:, c, j * P:(j + 1) * P],
                                    qt[:, 2 * c:2 * c + 2, i * P:(i + 1) * P],
                                    start=(c == 0), stop=(c == 1))
                            ts_ = apool.tile([P, 2 * P], F32, tag="ts")
                            nc.scalar.activation(ts_[:], qk[:], AF.Tanh,
                                                 scale=1.0 / SOFT_CAP)
                            mi = None
                            if j == i - WTILES:
                                mi = 0
                            elif j == i and mode == "tril":
                                mi = 1
                            elif j == i + WTILES and mode == "ones":
                                mi = 2
                            if mi is not None:
                                nc.vector.tensor_tensor(
                                    ts_[:], ts_[:], msk_sb[:, mi, :], ALU.add)
                            es = espool.tile([P, 2 * P], BF16,
                                             tag=f"es{j - lo}")
                            nc.scalar.activation(es[:], ts_[:], AF.Exp,
                                                 scale=SOFT_CAP)
                            es_list.append(es)
                        # softmax denominators: dn[0, q] = sum_k es[k, q]
                        dn = dnps.tile([1, 2 * P], F32, tag="dn")
                        for jj, es in enumerate(es_list):
                            nc.tensor.matmul(dn[:], ones_col[:], es[:],
                                             start=(jj == 0),
                                             stop=(jj == nj - 1))
                        rinv = apool.tile([1, 2 * P], F32, tag="rinv")
                        nc.vector.reciprocal_approx_fast(rinv[:], dn[:])
                        rb = rbpool.tile([P, 2 * P], F32, tag="rb")
                        nc.gpsimd.partition_broadcast(rb[:], rinv[:])
                        # PV: encT chunks [hd_c, (h0|h1) q]; enc's column
                        # order (h q) equals encsb chunk pair (c*2+h).
                        encsb = apool.tile([P, 4 * P], BF16, tag="encsb")
                        for c in range(2):
                            enc = encps.tile([P, 2 * P], F32, tag=f"enc{c}")
                            for jj, es in enumerate(es_list):
                                j = lo + jj
                                nc.tensor.matmul(
                                    enc[:],
                                    vsb[:, j, c * P:(c + 1) * P],
                                    es[:],
                                    start=(jj == 0), stop=(jj == nj - 1))
                            nc.vector.tensor_tensor(
                                encsb[:, 2 * c * P:(2 * c + 2) * P],
                                enc[:], rb[:], ALU.mult)
                        gtok = b * T + i * P
                        jb = gtok // TPC
                        ch = (gtok % TPC) // CHTOK
                        toff = gtok % CHTOK
                        # encsb chunk order (c*2+h) -> a2a order (h*2+c):
                        # permutation expressed on the DRAM-side AP; one
                        # DMA per c-chunk to stay within 3 balanced dims.
                        dstv = a2a_in[ch][jb].rearrange(
                            "(h c p) t -> p c h t", h=2, c=2)
                        for c in range(2):
                            nc.sync.dma_start(
                                dstv[:, c, :, toff:toff + P],
                                encsb[:, 2 * c * P:(2 * c + 2) * P].rearrange(
                                    "p (h t) -> p h t", h=2))

                    for ch in range(2):
                        for b in range(B):
                            for i in range(TQ):
                                if (i % 4) // 2 == ch:
                                    attend(b, i)
                        nc.gpsimd.collective_compute(
                            "AllToAll", ALU.bypass,
                            replica_groups=[list(range(NCORES))],
                            ins=[a2a_in[ch][:].opt()],
                            outs=[a2a_out[ch][:].opt()])
                        if ch == 0:
                            load_ef(0)
                            load_wo(0, 0)
                            load_wo(0, 1)

                # ==================== output projection ====================
                with tc.tile_pool(name="oproj_ps", bufs=2,
                                  space="PSUM") as opsum:
                    load_ef(1)
                    for half in range(2):
                        ef = efs[half]
                        for dp in range(D // DP):
                            if (half, dp) not in wo_tiles:
                                load_wo(half, dp)
                            wo_sb = wo_tiles[(half, dp)]
                            for t2 in range(2):
                                tc4 = half * 2 + t2
                                po = opsum.tile([P, DP], F32, tag="po")
                                for hc in range(NHC):
                                    nc.tensor.matmul(
                                        po[:],
                                        ef[:, hc, t2 * P:(t2 + 1) * P],
                                        wo_sb[:, hc, :],
                                        start=(hc == 0), stop=(hc == NHC - 1))
                                osb = ospool.tile([P, DP], F32, tag="osb")
                                nc.vector.tensor_copy(out=osb[:], in_=po[:])
                                nc.sync.dma_start(
                                    out[tc4 * P:(tc4 + 1) * P,
                                        dp * DP:(dp + 1) * DP],
                                    osb[:])

    nc.compile()
    return nc


def _rope_tables(pos, scale):
    """pos: [T] int array -> [2, 128, T] bf16 (cos;sin), scaled."""
    frac = 2.0 * np.arange(H // 2, dtype=np.float64) / H
    timescale = ROPE_BASE ** frac                      # [128]
    sinusoid = pos.astype(np.float64)[None, :] / timescale[:, None]  # [128,T]
    tabs = np.stack([np.cos(sinusoid), np.sin(sinusoid)]) * scale
    return tabs.astype(ml_dtypes.bfloat16)


def _masks():
    """[3, 128, 256] f32 additive masks in [k, q(2 heads)] layout."""
    kl = np.arange(P)[:, None]
    ql = np.arange(2 * P)[None, :] % P
    m0 = np.where(kl > ql, 0.0, MASKVAL)    # j == i-8: valid k_l > q_l
    m1 = np.where(kl <= ql, 0.0, MASKVAL)   # j == i (causal): valid k_l <= q_l
    m2 = np.where(kl < ql, 0.0, MASKVAL)    # j == i+8: valid k_l < q_l
    return np.stack([m0, m1, m2]).astype(np.float32)


def _reference_host(x, segment_pos, attn_mask, w_q, w_kv, w_o):
    """Slow but fully general fallback (numpy)."""
    xb = x.astype(np.float32)
    q = np.einsum('btd,ndh->btnh', xb, w_q)
    k = np.einsum('bsd,kdh->bskh', xb, w_kv[0])
    v = np.einsum('bsd,kdh->bskh', xb, w_kv[1])

    def rope(t, positions):
        hd = t.shape[-1]
        frac = 2.0 * np.arange(hd // 2, dtype=np.float32) / hd
        ts_ = ROPE_BASE ** frac
        sinusoid = positions.astype(np.float32)[..., None] / ts_
        sinusoid = sinusoid[..., None, :]
        s, c = np.sin(sinusoid), np.cos(sinusoid)
        first, second = np.split(t, 2, axis=-1)
        return np.concatenate([first * c - second * s,
                               second * c + first * s], axis=-1)

    q = rope(q, segment_pos) * SCALAR
    k = rope(k, segment_pos)
    qg = q.reshape(B, T, NKV, 2, H)
    logits = np.einsum('btkgh,bskh->btkgs', qg, k).reshape(B, T, NQ, T)
    logits = np.tanh(logits / SOFT_CAP) * SOFT_CAP
    pos_s = np.arange(T)[None, None, :]
    pos_t = segment_pos[:, :, None]
    sliding = (pos_s > pos_t - WINDOW) & (pos_s < pos_t + WINDOW)
    mask = np.logical_and(attn_mask, sliding)
    padded = np.where(mask[:, :, None, :], logits, -np.inf)
    padded -= padded.max(axis=-1, keepdims=True)
    e = np.exp(padded)
    probs = (e / e.sum(axis=-1, keepdims=True)).astype(np.float32)
    v_exp = np.repeat(v, NQ // NKV, axis=2)            # [B,T,NQ,H]
    enc = np.einsum('btns,bsnh->btnh', probs, v_exp)
    return np.einsum('btnh,nhd->btd', enc, w_o).astype(np.float32)


_GRAPH_CACHE = {}


def kernel(x, segment_pos, attn_mask, w_q, w_kv, w_o):
    global last_result
    x = np.asarray(x)
    segment_pos = np.asarray(segment_pos)
    attn_mask = np.asarray(attn_mask)
    w_q = np.asarray(w_q, dtype=np.float32)
    w_kv = np.asarray(w_kv, dtype=np.float32)
    w_o = np.asarray(w_o, dtype=np.float32)

    arange = np.broadcast_to(np.arange(T, dtype=segment_pos.dtype), (B, T))
    std_pos = np.array_equal(segment_pos, arange)
    tril = np.broadcast_to(np.tril(np.ones((T, T), dtype=bool)), (B, T, T))
    if attn_mask.all():
        mode = "ones"
    elif np.array_equal(attn_mask, tril):
        mode = "tril"
    else:
        mode = None
    if not std_pos or mode is None:
        return _reference_host(x, segment_pos, attn_mask, w_q, w_kv, w_o)

    if mode not in _GRAPH_CACHE:
        _GRAPH_CACHE[mode] = build(mode)
    nc = _GRAPH_CACHE[mode]

    bf = ml_dtypes.bfloat16
    xT = np.ascontiguousarray(x.reshape(TOK, D).T).astype(bf)    # [D, TOK]
    pos = segment_pos[0]
    ropeq = np.ascontiguousarray(_rope_tables(pos, SCALAR))
    ropek = np.ascontiguousarray(_rope_tables(pos, 1.0))
    wo_all = np.ascontiguousarray(
        w_o.reshape(NHC, P, D)).astype(bf)
    msk = np.ascontiguousarray(_masks())

    in_maps = []
    for c in range(NCORES):
        # wq columns in qt chunk order (c*2 + h): [c0h0, c0h1, c1h0, c1h1]
        g0, g1 = w_q[2 * c], w_q[2 * c + 1]
        wq_c = np.ascontiguousarray(
            np.concatenate([g0[:, :P], g1[:, :P], g0[:, P:], g1[:, P:]],
                           axis=1)).astype(bf)
        wk_c = np.ascontiguousarray(w_kv[0, c]).astype(bf)
        wv_c = np.ascontiguousarray(w_kv[1, c]).astype(bf)
        in_maps.append({
            "xT": xT, "wq": wq_c, "wk": wk_c, "wv": wv_c, "wo": wo_all,
            "ropeq": ropeq, "ropek": ropek, "msk": msk,
        })

    trace = os.environ.get("KTRACE", "0") == "1"
    res = run_bass_kernel_spmd(nc, in_maps, core_ids=list(range(NCORES)),
                               trace=trace)
    last_result = res
    outs = [res.results[c]["out"] for c in range(NCORES)]
    return np.concatenate(outs, axis=0).reshape(B, T, D).astype(np.float32)


# revision 19
# speedup vs baseline: 1.0876x; 1.0191x over previous
"""Sliding-window GQA attention (Gemma-style) on 8 TRN2 NeuronCores.

Sharding: tensor-parallel over heads. Core c owns q-heads {2c, 2c+1} and
kv-head c. Each core computes Q/K/V projections (+RoPE) for its heads over
the full sequence, banded sliding-window attention, then an AllToAll
(split into 2 token-chunks) reshards the attention output by token so
every core computes the full output projection for its 512-token slice.
Host concatenates slices.

Attention uses a [k, q] logits layout (K stationary, Q moving) so the
softmax probabilities come out of the Scalar engine directly in the
[k, q] orientation PV needs -- no PE transposes and no per-strip
normalization passes on the Vector engine. Row sums (softmax
denominators) are computed with ones-vector matmuls accumulated in PSUM;
the normalization is applied once on the (much smaller) attention output
via a broadcast matmul + Reciprocal activation.

All matmuls run in bf16 (f32 PSUM accumulation); softmax runs in f32.
"""

import os
import sys

for _p in ("/opt/trn_rl_repo",):
    if _p not in sys.path:
        sys.path.insert(0, _p)

import numpy as np
import ml_dtypes

import concourse.bass as bass
import concourse.mybir as mybir
import concourse.tile as tile
from concourse import bacc
from concourse.bass_utils import run_bass_kernel_spmd

F32 = mybir.dt.float32
BF16 = mybir.dt.bfloat16
AF = mybir.ActivationFunctionType
ALU = mybir.AluOpType

B, T, D = 2, 2048, 3584
NQ, NKV, H = 16, 8, 256
SCALAR = 0.0625
SOFT_CAP = 50.0
WINDOW = 1024
ROPE_BASE = 10000.0

NCORES = 8
P = 128
DC = D // P              # 28 contraction chunks
TQ = T // P              # 16 query tiles per batch
TPIECE = 256             # projection output tile width
NPIECE = T // TPIECE
HLOC = 2 * H             # 512 local q-head columns per core
TOK = B * T              # 4096
TPC = TOK // NCORES      # 512 tokens per core after AllToAll
CHTOK = TPC // 2         # 256 tokens per A2A chunk block
WTILES = WINDOW // P     # 8
MASKVAL = -1.0e30        # added to tanh output; exp(50*(t+MASKVAL)) == 0
NHC = NQ * H // P        # 32 global h chunks
DP = 512                 # output projection d piece

last_result = None       # BassKernelResults of the most recent device run


def _band(i, mode):
    lo = max(0, i - WTILES)
    hi = i if mode == "tril" else min(TQ - 1, i + WTILES)
    return lo, hi


def build(mode):
    assert mode in ("tril", "ones")
    nc = bacc.Bacc("TRN2", target_bir_lowering=False, debug=False,
                   num_devices=NCORES)

    xT = nc.dram_tensor("xT", [D, TOK], BF16, kind="ExternalInput")
    wq = nc.dram_tensor("wq", [D, HLOC], BF16, kind="ExternalInput")
    wk = nc.dram_tensor("wk", [D, H], BF16, kind="ExternalInput")
    wv = nc.dram_tensor("wv", [D, H], BF16, kind="ExternalInput")
    wo = nc.dram_tensor("wo", [NHC, P, D], BF16, kind="ExternalInput")
    ropeq = nc.dram_tensor("ropeq", [2, P, T], BF16, kind="ExternalInput")
    ropek = nc.dram_tensor("ropek", [2, P, T], BF16, kind="ExternalInput")
    msk = nc.dram_tensor("msk", [3, P, 2 * P], F32, kind="ExternalInput")
    out = nc.dram_tensor("out", [TPC, D], F32, kind="ExternalOutput")

    with tile.TileContext(nc) as tc:
        with (
            tc.tile_pool(name="dram", bufs=1, space="DRAM") as dram,
            tc.tile_pool(name="consts", bufs=1) as consts,
            tc.tile_pool(name="qkv", bufs=1) as qkvpool,
        ):
            # A2A bounce buffers: [src_rank_block][local h chunk-major][tok]
            a2a_in = [dram.tile([NCORES, HLOC, CHTOK], BF16,
                                name=f"a2a_in{m}") for m in range(2)]
            a2a_out = [dram.tile([NCORES, HLOC, CHTOK], BF16,
                                 name=f"a2a_out{m}") for m in range(2)]

            # ---- constants ----
            ones_col = consts.tile([P, 1], BF16)
            nc.gpsimd.memset(ones_col, 1.0)
            msk_sb = consts.tile([P, 3, 2 * P], F32)

            qts, kts, vsbs = [], [], []
            xT_v = xT[:].rearrange("(c p) t -> p c t", p=P)

            # =================== projections + rope =======================
            # qt chunk order is (c*2 + h): [c0h0, c0h1, c1h0, c1h1] so the
            # QK matmul rhs for contraction chunk c is a contiguous slice.
            with (
                tc.tile_pool(name="weights", bufs=1) as wpool,
                tc.tile_pool(name="xt", bufs=2) as xpool,
                tc.tile_pool(name="rtmp", bufs=2) as rpool,
                tc.tile_pool(name="proj_ps", bufs=4, space="PSUM") as ppsum,
                tc.tile_pool(name="projv_ps", bufs=2, space="PSUM") as vpsum,
            ):
                # Staged loads: first slices of wq and of the first x piece
                # go out first so the first matmuls start within a few us.
                wq_sb = wpool.tile([P, DC, HLOC], BF16)
                wq_v = wq[:].rearrange("(c p) h -> p c h", p=P)
                nc.sync.dma_start(wq_sb[:, 0:4, :], wq_v[:, 0:4, :])
                xt00 = xpool.tile([P, DC, TPIECE], BF16, tag="xt")
                nc.sync.dma_start(xt00[:, 0:4, :], xT_v[:, 0:4, 0:TPIECE])
                for d0, d1 in ((4, 12), (12, 20), (20, DC)):
                    nc.sync.dma_start(wq_sb[:, d0:d1, :], wq_v[:, d0:d1, :])
                nc.sync.dma_start(xt00[:, 4:, :], xT_v[:, 4:, 0:TPIECE])
                rq_sb = wpool.tile([P, 2, T], BF16)
                nc.sync.dma_start(rq_sb[:], ropeq[:].rearrange("s p t -> p s t"))
                wk_sb = wpool.tile([P, DC, H], BF16)
                nc.sync.dma_start(wk_sb[:],
                                  wk[:].rearrange("(c p) h -> p c h", p=P))
                wv_sb = wpool.tile([P, DC, H], BF16)
                nc.sync.dma_start(wv_sb[:],
                                  wv[:].rearrange("(c p) h -> p c h", p=P))
                rk_sb = wpool.tile([P, 2, T], BF16)
                nc.sync.dma_start(rk_sb[:], ropek[:].rearrange("s p t -> p s t"))
                nc.sync.dma_start(msk_sb[:], msk[:].rearrange("m p q -> p m q"))

                for b in range(B):
                    qt = qkvpool.tile([P, 4, T], BF16, tag=f"qt{b}")
                    kt = qkvpool.tile([P, 2, T], BF16, tag=f"kt{b}")
                    vsb = qkvpool.tile([P, TQ, H], BF16, tag=f"v{b}")
                    qts.append(qt)
                    kts.append(kt)
                    vsbs.append(vsb)

                    for pi in range(NPIECE):
                        t0 = pi * TPIECE
                        if b == 0 and pi == 0:
                            xt = xt00
                        else:
                            xt = xpool.tile([P, DC, TPIECE], BF16, tag="xt")
                            nc.sync.dma_start(
                                xt[:],
                                xT_v[:, :, b * T + t0:b * T + t0 + TPIECE])

                        def rope(dst, i0, i1, psA, psB, tab, t0=t0):
                            cos = tab[:, 0, t0:t0 + TPIECE]
                            sin = tab[:, 1, t0:t0 + TPIECE]
                            t1 = rpool.tile([P, TPIECE], F32, tag="r1")
                            t2 = rpool.tile([P, TPIECE], F32, tag="r2")
                            nc.vector.tensor_tensor(t1[:], psA[:], cos, ALU.mult)
                            nc.vector.tensor_tensor(t2[:], psB[:], sin, ALU.mult)
                            nc.vector.tensor_tensor(
                                dst[:, i0, t0:t0 + TPIECE], t1[:], t2[:],
                                ALU.subtract)
                            t3 = rpool.tile([P, TPIECE], F32, tag="r3")
                            t4 = rpool.tile([P, TPIECE], F32, tag="r4")
                            nc.vector.tensor_tensor(t3[:], psB[:], cos, ALU.mult)
                            nc.vector.tensor_tensor(t4[:], psA[:], sin, ALU.mult)
                            nc.vector.tensor_tensor(
                                dst[:, i1, t0:t0 + TPIECE], t3[:], t4[:],
                                ALU.add)

                        # Q projections: head h -> chunks h (c0) and 2+h (c1)
                        for hh in range(2):
                            ps = []
                            for cc in range(2):
                                hc = cc * 2 + hh
                                pq = ppsum.tile([P, TPIECE], F32, tag="pq")
                                for dc in range(DC):
                                    nc.tensor.matmul(
                                        pq[:],
                                        wq_sb[:, dc, hc * P:(hc + 1) * P],
                                        xt[:, dc, :],
                                        start=(dc == 0), stop=(dc == DC - 1))
                                ps.append(pq)
                            rope(qt, hh, 2 + hh, ps[0], ps[1], rq_sb)
                        # K projection: 2 h-chunks
                        ps = []
                        for half in range(2):
                            pk = ppsum.tile([P, TPIECE], F32, tag="pq")
                            for dc in range(DC):
                                nc.tensor.matmul(
                                    pk[:],
                                    wk_sb[:, dc, half * P:(half + 1) * P],
                                    xt[:, dc, :],
                                    start=(dc == 0), stop=(dc == DC - 1))
                            ps.append(pk)
                        rope(kt, 0, 1, ps[0], ps[1], rk_sb)
                        # V projection: natural layout [t, h]
                        for tc4 in range(TPIECE // P):
                            pv = vpsum.tile([P, H], F32, tag="pv")
                            for dc in range(DC):
                                nc.tensor.matmul(
                                    pv[:],
                                    xt[:, dc, tc4 * P:(tc4 + 1) * P],
                                    wv_sb[:, dc, :],
                                    start=(dc == 0), stop=(dc == DC - 1))
                            nc.scalar.activation(
                                vsb[:, pi * (TPIECE // P) + tc4, :],
                                pv[:], AF.Copy)

            # ============ banded attention in 2 rounds + A2A ==============
            # Output projection SBUF pools open early so wo prefetch can
            # start during the attention phase.
            with (
                tc.tile_pool(name="wo_sb", bufs=2 if mode == "tril" else 1) as wopool,
                tc.tile_pool(name="ef", bufs=1) as efpool,
                tc.tile_pool(name="osb", bufs=3) as ospool,
            ):
                wo_tiles = {}

                def load_wo(half, dp):
                    t = wopool.tile([P, NHC, DP], BF16, tag="wo")
                    nc.gpsimd.dma_start(
                        t[:],
                        wo[:, :, dp * DP:(dp + 1) * DP].rearrange(
                            "c p d -> p c d"))
                    wo_tiles[(half, dp)] = t

                efs = {}

                def load_ef(half):
                    ef = efpool.tile([P, NHC, CHTOK], BF16, tag=f"ef{half}")
                    nc.gpsimd.dma_start(
                        ef[:],
                        a2a_out[half][:].rearrange("b (c p) t -> p (b c) t",
                                                   p=P))
                    efs[half] = ef

                with (
                    tc.tile_pool(name="attn_sb", bufs=2) as apool,
                    tc.tile_pool(name="es_sb", bufs=2) as espool,
                    tc.tile_pool(name="rb_sb", bufs=2) as rbpool,
                    tc.tile_pool(name="qk_ps", bufs=3, space="PSUM") as qkps,
                    tc.tile_pool(name="dn_ps", bufs=2, space="PSUM") as dnps,
                    tc.tile_pool(name="enc_ps", bufs=1, space="PSUM") as encps,
                ):
                    SEGJ = 9   # max band tiles per softmax strip

                    def attend(b, i):
                        qt, kt, vsb = qts[b], kts[b], vsbs[b]
                        lo, hi = _band(i, mode)
                        nj = hi - lo + 1
                        nseg = (nj + SEGJ - 1) // SEGJ
                        es_slices = []
                        for si in range(nseg):
                            j0 = lo + si * SEGJ
                            j1 = min(j0 + SEGJ - 1, hi)
                            w = (j1 - j0 + 1) * 2 * P
                            ts_ = apool.tile([P, SEGJ * 2 * P], F32,
                                             tag=f"ts{si}")
                            for jj, j in enumerate(range(j0, j1 + 1)):
                                qk = qkps.tile([P, 2 * P], F32, tag="qk")
                                for c in range(2):
                                    nc.tensor.matmul(
                                        qk[:],
                                        kt[:, c, j * P:(j + 1) * P],
                                        qt[:, 2 * c:2 * c + 2,
                                           i * P:(i + 1) * P],
                                        start=(c == 0), stop=(c == 1))
                                nc.vector.tensor_copy(
                                    out=ts_[:, jj * 2 * P:(jj + 1) * 2 * P],
                                    in_=qk[:])
                            nc.scalar.activation(ts_[:, :w], ts_[:, :w],
                                                 AF.Tanh,
                                                 scale=1.0 / SOFT_CAP)
                            for jj, j in enumerate(range(j0, j1 + 1)):
                                mi = None
                                if j == i - WTILES:
                                    mi = 0
                                elif j == i and mode == "tril":
                                    mi = 1
                                elif j == i + WTILES and mode == "ones":
                                    mi = 2
                                if mi is not None:
                                    sl = slice(jj * 2 * P, (jj + 1) * 2 * P)
                                    nc.vector.tensor_tensor(
                                        ts_[:, sl], ts_[:, sl],
                                        msk_sb[:, mi, :], ALU.add)
                            es = espool.tile([P, SEGJ * 2 * P], BF16,
                                             tag=f"es{si}")
                            nc.scalar.activation(es[:, :w], ts_[:, :w],
                                                 AF.Exp, scale=SOFT_CAP)
                            for jj in range(j1 - j0 + 1):
                                es_slices.append(
                                    es[:, jj * 2 * P:(jj + 1) * 2 * P])
                        # softmax denominators: dn[0, q] = sum_k es[k, q]
                        dn = dnps.tile([1, 2 * P], F32, tag="dn")
                        for jj, esl in enumerate(es_slices):
                            nc.tensor.matmul(dn[:], ones_col[:], esl,
                                             start=(jj == 0),
                                             stop=(jj == nj - 1))
                        rinv = apool.tile([1, 2 * P], F32, tag="rinv")
                        nc.vector.reciprocal_approx_fast(rinv[:], dn[:])
                        rb = rbpool.tile([P, 2 * P], F32, tag="rb")
                        nc.gpsimd.partition_broadcast(rb[:], rinv[:])
                        # PV: encT chunks [hd_c, (h0|h1) q]; encsb stored
                        # directly in a2a chunk order (h*2+c) so a single
                        # 3D DMA ships it.
                        encsb = apool.tile([P, 4, P], BF16, tag="encsb")
                        encsb_v = encsb[:].rearrange("p (h c) t -> p c h t",
                                                     h=2)
                        for c in range(2):
                            enc = encps.tile([P, 2 * P], F32, tag=f"enc{c}")
                            for jj, esl in enumerate(es_slices):
                                j = lo + jj
                                nc.tensor.matmul(
                                    enc[:],
                                    vsb[:, j, c * P:(c + 1) * P],
                                    esl,
                                    start=(jj == 0), stop=(jj == nj - 1))
                            nc.vector.tensor_tensor(
                                encsb_v[:, c],
                                enc[:].rearrange("p (h t) -> p h t", h=2),
                                rb[:].rearrange("p (h t) -> p h t", h=2),
                                ALU.mult)
                        gtok = b * T + i * P
                        jb = gtok // TPC
                        ch = (gtok % TPC) // CHTOK
                        toff = gtok % CHTOK
                        dst = a2a_in[ch][jb].rearrange(
                            "(c p) t -> p c t", p=P)[:, :, toff:toff + P]
                        nc.sync.dma_start(dst, encsb[:])

                    for ch in range(2):
                        for b in range(B):
                            for i in range(TQ):
                                if (i % 4) // 2 == ch:
                                    attend(b, i)
                        nc.gpsimd.collective_compute(
                            "AllToAll", ALU.bypass,
                            replica_groups=[list(range(NCORES))],
                            ins=[a2a_in[ch][:].opt()],
                            outs=[a2a_out[ch][:].opt()])
                        if ch == 0:
                            load_ef(0)
                            load_wo(0, 0)
                            load_wo(0, 1)

                # ==================== output projection ====================
                with tc.tile_pool(name="oproj_ps", bufs=2,
                                  space="PSUM") as opsum:
                    for half in range(2):
                        if half == 1:
                            load_ef(1)
                        ef = efs[half]
                        for dp in range(D // DP):
                            if (half, dp) not in wo_tiles:
                                load_wo(half, dp)
                            wo_sb = wo_tiles[(half, dp)]
                            for t2 in range(2):
                                tc4 = half * 2 + t2
                                po = opsum.tile([P, DP], F32, tag="po")
                                for hc in range(NHC):
                                    nc.tensor.matmul(
                                        po[:],
                                        ef[:, hc, t2 * P:(t2 + 1) * P],
                                        wo_sb[:, hc, :],
                                        start=(hc == 0), stop=(hc == NHC - 1))
                                osb = ospool.tile([P, DP], F32, tag="osb")
                                nc.scalar.activation(osb[:], po[:], AF.Copy)
                                nc.sync.dma_start(
                                    out[tc4 * P:(tc4 + 1) * P,
                                        dp * DP:(dp + 1) * DP],
                                    osb[:])

    nc.compile()
    return nc


def _rope_tables(pos, scale):
    """pos: [T] int array -> [2, 128, T] bf16 (cos;sin), scaled."""
    frac = 2.0 * np.arange(H // 2, dtype=np.float64) / H
    timescale = ROPE_BASE ** frac                      # [128]
    sinusoid = pos.astype(np.float64)[None, :] / timescale[:, None]  # [128,T]
    tabs = np.stack([np.cos(sinusoid), np.sin(sinusoid)]) * scale
    return tabs.astype(ml_dtypes.bfloat16)


def _masks():
    """[3, 128, 256] f32 additive masks in [k, q(2 heads)] layout."""
    kl = np.arange(P)[:, None]
    ql = np.arange(2 * P)[None, :] % P
    m0 = np.where(kl > ql, 0.0, MASKVAL)    # j == i-8: valid k_l > q_l
    m1 = np.where(kl <= ql, 0.0, MASKVAL)   # j == i (causal): valid k_l <= q_l
    m2 = np.where(kl < ql, 0.0, MASKVAL)    # j == i+8: valid k_l < q_l
    return np.stack([m0, m1, m2]).astype(np.float32)


def _reference_host(x, segment_pos, attn_mask, w_q, w_kv, w_o):
    """Slow but fully general fallback (numpy)."""
    xb = x.astype(np.float32)
    q = np.einsum('btd,ndh->btnh', xb, w_q)
    k = np.einsum('bsd,kdh->bskh', xb, w_kv[0])
    v = np.einsum('bsd,kdh->bskh', xb, w_kv[1])

    def rope(t, positions):
        hd = t.shape[-1]
        frac = 2.0 * np.arange(hd // 2, dtype=np.float32) / hd
        ts_ = ROPE_BASE ** frac
        sinusoid = positions.astype(np.float32)[..., None] / ts_
        sinusoid = sinusoid[..., None, :]
        s, c = np.sin(sinusoid), np.cos(sinusoid)
        first, second = np.split(t, 2, axis=-1)
        return np.concatenate([first * c - second * s,
                               second * c + first * s], axis=-1)

    q = rope(q, segment_pos) * SCALAR
    k = rope(k, segment_pos)
    qg = q.reshape(B, T, NKV, 2, H)
    logits = np.einsum('btkgh,bskh->btkgs', qg, k).reshape(B, T, NQ, T)
    logits = np.tanh(logits / SOFT_CAP) * SOFT_CAP
    pos_s = np.arange(T)[None, None, :]
    pos_t = segment_pos[:, :, None]
    sliding = (pos_s > pos_t - WINDOW) & (pos_s < pos_t + WINDOW)
    mask = np.logical_and(attn_mask, sliding)
    padded = np.where(mask[:, :, None, :], logits, -np.inf)
    padded -= padded.max(axis=-1, keepdims=True)
    e = np.exp(padded)
    probs = (e / e.sum(axis=-1, keepdims=True)).astype(np.float32)
    v_exp = np.repeat(v, NQ // NKV, axis=2)            # [B,T,NQ,H]
    enc = np.einsum('btns,bsnh->btnh', probs, v_exp)
    return np.einsum('btnh,nhd->btd', enc, w_o).astype(np.float32)


_GRAPH_CACHE = {}


def kernel(x, segment_pos, attn_mask, w_q, w_kv, w_o):
    global last_result
    x = np.asarray(x)
    segment_pos = np.asarray(segment_pos)
    attn_mask = np.asarray(attn_mask)
    w_q = np.asarray(w_q, dtype=np.float32)
    w_kv = np.asarray(w_kv, dtype=np.float32)
    w_o = np.asarray(w_o, dtype=np.float32)

    arange = np.broadcast_to(np.arange(T, dtype=segment_pos.dtype), (B, T))
    std_pos = np.array_equal(segment_pos, arange)
    tril = np.broadcast_to(np.tril(np.ones((T, T), dtype=bool)), (B, T, T))
    if attn_mask.all():
        mode = "ones"
    elif np.array_equal(attn_mask, tril):
        mode = "tril"
    else:
        mode = None
    if not std_pos or mode is None:
        return _reference_host(x, segment_pos, attn_mask, w_q, w_kv, w_o)

    if mode not in _GRAPH_CACHE:
        _GRAPH_CACHE[mode] = build(mode)
    nc = _GRAPH_CACHE[mode]

    bf = ml_dtypes.bfloat16
    xT = np.ascontiguousarray(x.reshape(TOK, D).T).astype(bf)    # [D, TOK]
    pos = segment_pos[0]
    ropeq = np.ascontiguousarray(_rope_tables(pos, SCALAR))
    ropek = np.ascontiguousarray(_rope_tables(pos, 1.0))
    wo_all = np.ascontiguousarray(
        w_o.reshape(NHC, P, D)).astype(bf)
    msk = np.ascontiguousarray(_masks())

    in_maps = []
    for c in range(NCORES):
        # wq columns in qt chunk order (c*2 + h): [c0h0, c0h1, c1h0, c1h1]
        g0, g1 = w_q[2 * c], w_q[2 * c + 1]
        wq_c = np.ascontiguousarray(
            np.concatenate([g0[:, :P], g1[:, :P], g0[:, P:], g1[:, P:]],
                           axis=1)).astype(bf)
        wk_c = np.ascontiguousarray(w_kv[0, c]).astype(bf)
        wv_c = np.ascontiguousarray(w_kv[1, c]).astype(bf)
        in_maps.append({
            "xT": xT, "wq": wq_c, "wk": wk_c, "wv": wv_c, "wo": wo_all,
            "ropeq": ropeq, "ropek": ropek, "msk": msk,
        })

    trace = os.environ.get("KTRACE", "0") == "1"
    res = run_bass_kernel_spmd(nc, in_maps, core_ids=list(range(NCORES)),
                               trace=trace)
    last_result = res
    outs = [res.results[c]["out"] for c in range(NCORES)]
    return np.concatenate(outs, axis=0).reshape(B, T, D).astype(np.float32)
